# revision 2
# baseline (speedup 1.0000x reference)
"""AFT-Local distributed Trainium2 kernel (8 NeuronCores).

Math (reference, with cancellations):
  q = query @ Wq.T; k = key_in @ Wk.T; v = value @ Wv.T      [S,B,D]
  E[i,j] = exp(pos_bias[i,j] * (j <= i-255))                 [S,S] (masked-out -> exp(0)=1)
  num[i,b,:] = sum_j E[i,j] * (exp(k)*v)[j,b,:]
  den[i,b,:] = sum_j E[i,j] *  exp(k)[j,b,:]
  out = (sigmoid(q) * num / den) @ Wo.T
The max-subtractions in the reference cancel in num/den, and all values are
small enough that plain exp is safe in f32/bf16.

Distribution: sequence-parallel. Core c owns output row-tiles {c, 15-c}
(128 rows each). Each core projects its own 256-row j-shard of k/v, applies
exp, and AllGathers bf16 ek / ek*v across the 8 cores; attention + q/o
projections then run fully local. All per-core variation (which i-rows,
masked pos-bias slice) is carried in the input data so the SPMD program is
identical on every core.

On-device layout notes ("T" suffix = transposed so the contraction dim lands
on SBUF partitions):
  keyT/valT/queryT [D, 1024] bf16 : col = b*256 + s_local
  w*               [D, D]   bf16 : host-pretransposed (W.T)
  pbT              [S, 256] bf16 : pos_bias[i_rows, :].T, host-masked
  out              [B, 2, 128, D] f32
"""

import os
import sys

import numpy as np
import ml_dtypes

sys.path.insert(0, "/opt/trn_rl_repo")

S, B, D, W = 2048, 4, 1024, 256
NC = 8
P = 128
NT = S // P  # 16 global i/j tiles

_CACHE = {}


def _build():
    import concourse.bass as bass
    import concourse.bacc as bacc
    import concourse.mybir as mybir
    import concourse.tile as tile

    f32 = mybir.dt.float32
    bf16 = mybir.dt.bfloat16
    AF = mybir.ActivationFunctionType

    nc = bacc.Bacc("TRN2", target_bir_lowering=False, debug=False, num_devices=NC)

    keyT = nc.dram_tensor("keyT", [D, 1024], bf16, kind="ExternalInput")
    valT = nc.dram_tensor("valT", [D, 1024], bf16, kind="ExternalInput")
    queryT = nc.dram_tensor("queryT", [D, 1024], bf16, kind="ExternalInput")
    pbT = nc.dram_tensor("pbT", [S, 256], bf16, kind="ExternalInput")
    wk = nc.dram_tensor("wk", [D, D], bf16, kind="ExternalInput")
    wv = nc.dram_tensor("wv", [D, D], bf16, kind="ExternalInput")
    wq = nc.dram_tensor("wq", [D, D], bf16, kind="ExternalInput")
    wo = nc.dram_tensor("wo", [D, D], bf16, kind="ExternalInput")
    out = nc.dram_tensor("out", [B, 2, P, D], f32, kind="ExternalOutput")

    with tile.TileContext(nc) as tc:
        with (
            tc.tile_pool(name="persist", bufs=1) as persist,
            tc.tile_pool(name="dram", bufs=1, space="DRAM") as dram,
        ):
            ag_in = dram.tile([1024, 2048], bf16)
            ag_out = dram.tile([NC * 1024, 2048], bf16, addr_space="Shared")

            # ---- persistent SBUF tensors ----
            wq_sb = []
            qT_sb = []
            wo_sb = []
            for kt in range(8):
                t1 = persist.tile([P, D], bf16, name=f"wq{kt}")
                nc.sync.dma_start(out=t1[:], in_=wq[kt * P : (kt + 1) * P, :])
                wq_sb.append(t1)
                t2 = persist.tile([P, 1024], bf16, name=f"qT{kt}")
                nc.sync.dma_start(out=t2[:], in_=queryT[kt * P : (kt + 1) * P, :])
                qT_sb.append(t2)
                t3 = persist.tile([P, D], bf16, name=f"wo{kt}")
                nc.sync.dma_start(out=t3[:], in_=wo[kt * P : (kt + 1) * P, :])
                wo_sb.append(t3)
            e_sb = [persist.tile([P, 256], bf16, name=f"e{jt}") for jt in range(NT)]
            sigq = [
                [persist.tile([P, 256], bf16, name=f"sq{b}_{et}") for et in range(8)]
                for b in range(B)
            ]
            yT = [
                [persist.tile([P, 256], bf16, name=f"yT{b}_{dt}") for dt in range(8)]
                for b in range(B)
            ]

            # ---- phase A: k/v projection on own j-shard, exp, stage for AG ----
            with (
                tc.tile_pool(name="pa", bufs=1) as pa,
                tc.tile_pool(name="pa_st", bufs=3) as pa_st,
                tc.tile_pool(name="ps_a", bufs=2, space="PSUM") as ps_a,
            ):
                keyT_sb = []
                valT_sb = []
                wk_sb = []
                wv_sb = []
                for kt in range(8):
                    a1 = pa.tile([P, 1024], bf16, name=f"keyT{kt}")
                    nc.sync.dma_start(out=a1[:], in_=keyT[kt * P : (kt + 1) * P, :])
                    keyT_sb.append(a1)
                    a2 = pa.tile([P, 1024], bf16, name=f"valT{kt}")
                    nc.sync.dma_start(out=a2[:], in_=valT[kt * P : (kt + 1) * P, :])
                    valT_sb.append(a2)
                    a3 = pa.tile([P, D], bf16, name=f"wk{kt}")
                    nc.sync.dma_start(out=a3[:], in_=wk[kt * P : (kt + 1) * P, :])
                    wk_sb.append(a3)
                    a4 = pa.tile([P, D], bf16, name=f"wv{kt}")
                    nc.sync.dma_start(out=a4[:], in_=wv[kt * P : (kt + 1) * P, :])
                    wv_sb.append(a4)

                for tt in range(8):
                    for es in range(2):
                        psk = ps_a.tile([P, 512], f32, tag="psk")
                        psv = ps_a.tile([P, 512], f32, tag="psv")
                        for kt in range(8):
                            nc.tensor.matmul(
                                psk[:],
                                keyT_sb[kt][:, tt * P : (tt + 1) * P],
                                wk_sb[kt][:, es * 512 : (es + 1) * 512],
                                start=(kt == 0),
                                stop=(kt == 7),
                            )
                        for kt in range(8):
                            nc.tensor.matmul(
                                psv[:],
                                valT_sb[kt][:, tt * P : (tt + 1) * P],
                                wv_sb[kt][:, es * 512 : (es + 1) * 512],
                                start=(kt == 0),
                                stop=(kt == 7),
                            )
                        ekf = pa_st.tile([P, 512], f32, tag="ekf")
                        nc.scalar.activation(ekf[:], psk[:], AF.Exp)
                        ekb = pa_st.tile([P, 512], bf16, tag="ekb")
                        nc.vector.tensor_copy(ekb[:], ekf[:])
                        ekvb = pa_st.tile([P, 512], bf16, tag="ekvb")
                        nc.vector.tensor_mul(ekvb[:], ekf[:], psv[:])
                        nc.sync.dma_start(
                            out=ag_in[tt * P : (tt + 1) * P, es * 512 : (es + 1) * 512],
                            in_=ekb[:],
                        )
                        nc.sync.dma_start(
                            out=ag_in[
                                tt * P : (tt + 1) * P, 1024 + es * 512 : 1024 + (es + 1) * 512
                            ],
                            in_=ekvb[:],
                        )

            # ---- AllGather ek|ekv across the 8 cores ----
            nc.gpsimd.collective_compute(
                "AllGather",
                mybir.AluOpType.bypass,
                ins=[ag_in.opt()],
                outs=[ag_out.opt()],
                replica_groups=[list(range(NC))],
            )

            # ---- phase C: q projection + sigmoid (overlaps the AllGather) ----
            with tc.tile_pool(name="ps_c", bufs=2, space="PSUM") as ps_c:
                for b in range(B):
                    for et in range(8):
                        psq = ps_c.tile([P, 256], f32, tag="psq")
                        for kt in range(8):
                            nc.tensor.matmul(
                                psq[:],
                                wq_sb[kt][:, et * P : (et + 1) * P],
                                qT_sb[kt][:, b * 256 : (b + 1) * 256],
                                start=(kt == 0),
                                stop=(kt == 7),
                            )
                        nc.scalar.activation(sigq[b][et][:], psq[:], AF.Sigmoid)

            # ---- phase D: E = exp(host-masked pos_bias^T slice) ----
            with tc.tile_pool(name="pd", bufs=3) as pd:
                for jt in range(NT):
                    pbt = pd.tile([P, 256], bf16, tag="pbt")
                    nc.sync.dma_start(out=pbt[:], in_=pbT[jt * P : (jt + 1) * P, :])
                    nc.scalar.activation(e_sb[jt][:], pbt[:], AF.Exp)

            # ---- phase E: num/den = E^T-weighted sums over gathered ek/ekv ----
            with (
                tc.tile_pool(name="pe_st", bufs=4) as pe_st,
                tc.tile_pool(name="pe_ep", bufs=2) as pe_ep,
                tc.tile_pool(name="ps_e", bufs=2, space="PSUM") as ps_e,
            ):
                for b in range(B):
                    for p in range(4):  # dt pairs (2p, 2p+1)
                        num0 = ps_e.tile([P, 256], f32, tag="num0")
                        den0 = ps_e.tile([P, 256], f32, tag="den0")
                        num1 = ps_e.tile([P, 256], f32, tag="num1")
                        den1 = ps_e.tile([P, 256], f32, tag="den1")
                        for jt in range(NT):
                            r0 = 1024 * (jt // 2) + 256 * b + P * (jt % 2)
                            ekt = pe_st.tile([P, 256], bf16, tag="ekt")
                            nc.sync.dma_start(
                                out=ekt[:], in_=ag_out[r0 : r0 + P, 256 * p : 256 * (p + 1)]
                            )
                            ekvt = pe_st.tile([P, 256], bf16, tag="ekvt")
                            nc.sync.dma_start(
                                out=ekvt[:],
                                in_=ag_out[r0 : r0 + P, 1024 + 256 * p : 1024 + 256 * (p + 1)],
                            )
                            st = jt == 0
                            sp = jt == NT - 1
                            nc.tensor.matmul(
                                num0[:], ekvt[:, 0:P], e_sb[jt][:], start=st, stop=sp
                            )
                            nc.tensor.matmul(
                                den0[:], ekt[:, 0:P], e_sb[jt][:], start=st, stop=sp
                            )
                            nc.tensor.matmul(
                                num1[:], ekvt[:, P:256], e_sb[jt][:], start=st, stop=sp
                            )
                            nc.tensor.matmul(
                                den1[:], ekt[:, P:256], e_sb[jt][:], start=st, stop=sp
                            )
                        for h, (nm, dn) in enumerate(((num0, den0), (num1, den1))):
                            dt = 2 * p + h
                            rec = pe_ep.tile([P, 256], f32, tag="rec")
                            nc.vector.reciprocal(rec[:], dn[:])
                            tmp = pe_ep.tile([P, 256], f32, tag="tmp")
                            nc.vector.tensor_mul(tmp[:], nm[:], rec[:])
                            nc.vector.tensor_mul(yT[b][dt][:], tmp[:], sigq[b][dt][:])

            # ---- phase F: output projection ----
            with (
                tc.tile_pool(name="pf", bufs=3) as pf,
                tc.tile_pool(name="ps_f", bufs=2, space="PSUM") as ps_f,
            ):
                for b in range(B):
                    for ih in range(2):
                        for es in range(2):
                            pso = ps_f.tile([P, 512], f32, tag="pso")
                            for dt in range(8):
                                nc.tensor.matmul(
                                    pso[:],
                                    yT[b][dt][:, ih * P : (ih + 1) * P],
                                    wo_sb[dt][:, es * 512 : (es + 1) * 512],
                                    start=(dt == 0),
                                    stop=(dt == 7),
                                )
                            osb = pf.tile([P, 512], f32, tag="osb")
                            nc.scalar.activation(osb[:], pso[:], AF.Copy)
                            nc.sync.dma_start(
                                out=out[b, ih, :, es * 512 : (es + 1) * 512], in_=osb[:]
                            )

    nc.compile()
    return nc


def _prep_inputs(query, key_in, value, Wq, Wk, Wv, pos_bias):
    bf = ml_dtypes.bfloat16
    wq_t = np.ascontiguousarray(Wq.T).astype(bf)
    wk_t = np.ascontiguousarray(Wk.T).astype(bf)
    wv_t = np.ascontiguousarray(Wv.T).astype(bf)

    jj = np.arange(S)
    in_maps = []
    for c in range(NC):
        t0, t1 = c, NT - 1 - c
        i_idx = np.r_[t0 * P : (t0 + 1) * P, t1 * P : (t1 + 1) * P]
        pb = pos_bias[i_idx, :].T.copy()  # [S, 256]
        valid = jj[:, None] <= (i_idx[None, :] - (W - 1))
        pb[~valid] = 0.0

        sl = slice(256 * c, 256 * (c + 1))
        in_maps.append(
            {
                "keyT": np.ascontiguousarray(
                    key_in[sl].transpose(2, 1, 0).reshape(D, 1024)
                ).astype(bf),
                "valT": np.ascontiguousarray(
                    value[sl].transpose(2, 1, 0).reshape(D, 1024)
                ).astype(bf),
                "queryT": np.ascontiguousarray(
                    query[i_idx].transpose(2, 1, 0).reshape(D, 1024)
                ).astype(bf),
                "pbT": pb.astype(bf),
                "wk": wk_t,
                "wv": wv_t,
                "wq": wq_t,
                "wo": None,  # filled by caller (needs Wo)
            }
        )
    return in_maps


def _run(inputs, trace=False):
    from concourse.bass_utils import run_bass_kernel_spmd

    if "nc" not in _CACHE:
        _CACHE["nc"] = _build()
    nc = _CACHE["nc"]

    in_maps = _prep_inputs(
        inputs["query"],
        inputs["key_in"],
        inputs["value"],
        inputs["Wq"],
        inputs["Wk"],
        inputs["Wv"],
        inputs["pos_bias"],
    )
    wo_t = np.ascontiguousarray(inputs["Wo"].T).astype(ml_dtypes.bfloat16)
    for m in in_maps:
        m["wo"] = wo_t

    res = run_bass_kernel_spmd(nc, in_maps, core_ids=list(range(NC)), trace=trace)

    full = np.empty((S, B, D), np.float32)
    for c in range(NC):
        o = np.asarray(res.results[c]["out"], np.float32)  # [B, 2, P, D]
        for ih, t in enumerate((c, NT - 1 - c)):
            full[t * P : (t + 1) * P] = o[:, ih].transpose(1, 0, 2)
    return full, res


def kernel(**inputs):
    inputs = {k: np.asarray(v) for k, v in inputs.items()}
    full, _ = _run(inputs, trace=False)
    return full


if __name__ == "__main__":
    inputs = np.load("/tmp/inputs.npy", allow_pickle=True).item()
    out = kernel(**inputs)
    print("out", out.shape, out.dtype)


# revision 3
# speedup vs baseline: 1.3138x; 1.3138x over previous
"""AFT-Local distributed Trainium2 kernel (8 NeuronCores).

Math (reference, with cancellations):
  q = query @ Wq.T; k = key_in @ Wk.T; v = value @ Wv.T      [S,B,D]
  E[i,j] = exp(pos_bias[i,j] * (j <= i-255))                 [S,S]
  num[i,b,:] = sum_j E[i,j] * (exp(k)*v)[j,b,:]
  den[i,b,:] = sum_j E[i,j] *  exp(k)[j,b,:]
  out = (sigmoid(q) * num / den) @ Wo.T
The max-subtractions in the reference cancel in num/den; all values are small
enough that plain exp is safe.

Distribution: sequence-parallel. Core c owns output row-tiles {c, 15-c}
(128 rows each; paired so every core does identical work). Each core projects
its own 256-row j-shard of k/v, applies exp, and AllGathers bf16 ek / ek*v;
attention + q/o projections are then local. All per-core variation (which
i-rows, masked pos-bias slice) is carried in input data so the SPMD program
is identical on every core.

Perf structure (v2): every matmul phase uses long accumulation chains into a
single PSUM bank (HAM stays warm; LDWEIGHTS overlaps); attention streams
gathered ek/ekv as [128,1024] row-blocks (contiguous 2KB DMA rows); the
num/den epilogue produces y in [i,d]; a PE-transpose turns it into the
stationary operand for the output projection.
"""

import os
import sys

import numpy as np
import ml_dtypes

sys.path.insert(0, "/opt/trn_rl_repo")

S, B, D, W = 2048, 4, 1024, 256
NC = 8
P = 128
NT = S // P  # 16 global i/j tiles

_CACHE = {}


def _build():
    import concourse.bass as bass
    import concourse.bacc as bacc
    import concourse.mybir as mybir
    import concourse.tile as tile
    from concourse.masks import make_identity

    f32 = mybir.dt.float32
    bf16 = mybir.dt.bfloat16
    AF = mybir.ActivationFunctionType

    nc = bacc.Bacc("TRN2", target_bir_lowering=False, debug=False, num_devices=NC)

    keyT = nc.dram_tensor("keyT", [D, 1024], bf16, kind="ExternalInput")
    valT = nc.dram_tensor("valT", [D, 1024], bf16, kind="ExternalInput")
    queryT = nc.dram_tensor("queryT", [D, 1024], bf16, kind="ExternalInput")
    pbT = nc.dram_tensor("pbT", [S, 256], bf16, kind="ExternalInput")
    wk = nc.dram_tensor("wk", [D, D], bf16, kind="ExternalInput")
    wv = nc.dram_tensor("wv", [D, D], bf16, kind="ExternalInput")
    wq = nc.dram_tensor("wq", [D, D], bf16, kind="ExternalInput")
    wo = nc.dram_tensor("wo", [D, D], bf16, kind="ExternalInput")
    out = nc.dram_tensor("out", [B, 2, P, D], f32, kind="ExternalOutput")

    with tile.TileContext(nc) as tc:
        with (
            tc.tile_pool(name="persist", bufs=1) as persist,
            tc.tile_pool(name="dram", bufs=1, space="DRAM") as dram,
        ):
            ag_in = dram.tile([1024, 2048], bf16)
            ag_out = dram.tile([NC * 1024, 2048], bf16, addr_space="Shared")

            # ---- phase A: k/v projection on own j-shard, exp, stage for AG ----
            with (
                tc.tile_pool(name="pa", bufs=1) as pa,
                tc.tile_pool(name="pa_st", bufs=3) as pa_st,
                tc.tile_pool(name="ps_a", bufs=2, space="PSUM") as ps_a,
            ):
                keyT_sb = []
                valT_sb = []
                wk_sb = []
                wv_sb = []
                for kt in range(8):
                    a1 = pa.tile([P, 1024], bf16, name=f"keyT{kt}")
                    nc.sync.dma_start(out=a1[:], in_=keyT[kt * P : (kt + 1) * P, :])
                    keyT_sb.append(a1)
                    a2 = pa.tile([P, 1024], bf16, name=f"valT{kt}")
                    nc.sync.dma_start(out=a2[:], in_=valT[kt * P : (kt + 1) * P, :])
                    valT_sb.append(a2)
                    a3 = pa.tile([P, D], bf16, name=f"wk{kt}")
                    nc.sync.dma_start(out=a3[:], in_=wk[kt * P : (kt + 1) * P, :])
                    wk_sb.append(a3)
                    a4 = pa.tile([P, D], bf16, name=f"wv{kt}")
                    nc.sync.dma_start(out=a4[:], in_=wv[kt * P : (kt + 1) * P, :])
                    wv_sb.append(a4)

                for tt in range(8):
                    for es in range(2):
                        psk = ps_a.tile([P, 512], f32, tag="psk")
                        psv = ps_a.tile([P, 512], f32, tag="psv")
                        for kt in range(8):
                            nc.tensor.matmul(
                                psk[:],
                                keyT_sb[kt][:, tt * P : (tt + 1) * P],
                                wk_sb[kt][:, es * 512 : (es + 1) * 512],
                                start=(kt == 0),
                                stop=(kt == 7),
                            )
                        for kt in range(8):
                            nc.tensor.matmul(
                                psv[:],
                                valT_sb[kt][:, tt * P : (tt + 1) * P],
                                wv_sb[kt][:, es * 512 : (es + 1) * 512],
                                start=(kt == 0),
                                stop=(kt == 7),
                            )
                        ekf = pa_st.tile([P, 512], f32, tag="ekf")
                        nc.scalar.activation(ekf[:], psk[:], AF.Exp)
                        ekb = pa_st.tile([P, 512], bf16, tag="ekb")
                        nc.vector.tensor_copy(ekb[:], ekf[:])
                        ekvb = pa_st.tile([P, 512], bf16, tag="ekvb")
                        nc.vector.tensor_mul(ekvb[:], ekf[:], psv[:])
                        nc.sync.dma_start(
                            out=ag_in[tt * P : (tt + 1) * P, es * 512 : (es + 1) * 512],
                            in_=ekb[:],
                        )
                        nc.sync.dma_start(
                            out=ag_in[
                                tt * P : (tt + 1) * P,
                                1024 + es * 512 : 1024 + (es + 1) * 512,
                            ],
                            in_=ekvb[:],
                        )

            # ---- AllGather ek|ekv across the 8 cores ----
            nc.gpsimd.collective_compute(
                "AllGather",
                mybir.AluOpType.bypass,
                ins=[ag_in.opt()],
                outs=[ag_out.opt()],
                replica_groups=[list(range(NC))],
            )

            # ---- persistent tensors needed from phase C on ----
            wq_sb = []
            qT_sb = []
            wo_sb = []
            for kt in range(8):
                t1 = persist.tile([P, D], bf16, name=f"wq{kt}")
                nc.sync.dma_start(out=t1[:], in_=wq[kt * P : (kt + 1) * P, :])
                wq_sb.append(t1)
                t2 = persist.tile([P, 1024], bf16, name=f"qT{kt}")
                nc.sync.dma_start(out=t2[:], in_=queryT[kt * P : (kt + 1) * P, :])
                qT_sb.append(t2)
                t3 = persist.tile([P, D], bf16, name=f"wo{kt}")
                nc.sync.dma_start(out=t3[:], in_=wo[kt * P : (kt + 1) * P, :])
                wo_sb.append(t3)
            e_sb = [persist.tile([P, 256], bf16, name=f"e{jt}") for jt in range(NT)]
            sq_sb = [
                [persist.tile([P, 1024], bf16, name=f"sq{b}_{ih}") for ih in range(2)]
                for b in range(B)
            ]
            y_bf = [
                [persist.tile([P, 1024], bf16, name=f"y{b}_{ih}") for ih in range(2)]
                for b in range(B)
            ]
            ident = persist.tile([P, P], bf16, name="ident")
            make_identity(nc, ident[:])

            # ---- phase D: E = exp(host-masked pos_bias^T slice) ----
            with tc.tile_pool(name="pd", bufs=3) as pd:
                for jt in range(NT):
                    pbt = pd.tile([P, 256], bf16, tag="pbt")
                    nc.sync.dma_start(out=pbt[:], in_=pbT[jt * P : (jt + 1) * P, :])
                    nc.scalar.activation(e_sb[jt][:], pbt[:], AF.Exp)

            # ---- phase C: q projection + sigmoid (overlaps the AllGather) ----
            with tc.tile_pool(name="ps_c", bufs=2, space="PSUM") as ps_c:
                for b in range(B):
                    for ih in range(2):
                        it = b * 2 + ih
                        psqa = ps_c.tile([P, 512], f32, tag="psqa")
                        psqb = ps_c.tile([P, 512], f32, tag="psqb")
                        for kt in range(8):
                            nc.tensor.matmul(
                                psqa[:],
                                qT_sb[kt][:, it * P : (it + 1) * P],
                                wq_sb[kt][:, 0:512],
                                start=(kt == 0),
                                stop=(kt == 7),
                            )
                        for kt in range(8):
                            nc.tensor.matmul(
                                psqb[:],
                                qT_sb[kt][:, it * P : (it + 1) * P],
                                wq_sb[kt][:, 512:1024],
                                start=(kt == 0),
                                stop=(kt == 7),
                            )
                        nc.scalar.activation(sq_sb[b][ih][:, 0:512], psqa[:], AF.Sigmoid)
                        nc.scalar.activation(
                            sq_sb[b][ih][:, 512:1024], psqb[:], AF.Sigmoid
                        )

            # ---- phase E: num/den chains over gathered ek/ekv; y epilogue ----
            with (
                tc.tile_pool(name="pe_blk", bufs=1) as pe_blk,
                tc.tile_pool(name="pe_ep", bufs=2) as pe_ep,
                tc.tile_pool(name="ps_e", bufs=2, space="PSUM") as ps_e,
            ):
                for b in range(B):
                    ek_blk = []
                    ekv_blk = []
                    for jt in range(NT):
                        r0 = 1024 * (jt // 2) + 256 * b + P * (jt % 2)
                        e1 = pe_blk.tile([P, 1024], bf16, tag=f"ek{jt}", name=f"ek{jt}")
                        nc.sync.dma_start(out=e1[:], in_=ag_out[r0 : r0 + P, 0:1024])
                        ek_blk.append(e1)
                        e2 = pe_blk.tile(
                            [P, 1024], bf16, tag=f"ekv{jt}", name=f"ekv{jt}"
                        )
                        nc.sync.dma_start(
                            out=e2[:], in_=ag_out[r0 : r0 + P, 1024:2048]
                        )
                        ekv_blk.append(e2)
                    for ih in range(2):
                        isl = slice(ih * P, (ih + 1) * P)
                        na = ps_e.tile([P, 512], f32, tag="na")
                        nb = ps_e.tile([P, 512], f32, tag="nb")
                        da = ps_e.tile([P, 512], f32, tag="da")
                        db = ps_e.tile([P, 512], f32, tag="db")
                        for tgt, blk, c0 in (
                            (na, ekv_blk, 0),
                            (nb, ekv_blk, 512),
                            (da, ek_blk, 0),
                            (db, ek_blk, 512),
                        ):
                            for jt in range(NT):
                                nc.tensor.matmul(
                                    tgt[:],
                                    e_sb[jt][:, isl],
                                    blk[jt][:, c0 : c0 + 512],
                                    start=(jt == 0),
                                    stop=(jt == NT - 1),
                                )
                        for half, (nm, dn) in enumerate(((na, da), (nb, db))):
                            c0 = half * 512
                            rec = pe_ep.tile([P, 512], f32, tag="rec")
                            nc.vector.reciprocal(rec[:], dn[:])
                            tmp = pe_ep.tile([P, 512], f32, tag="tmp")
                            nc.vector.tensor_mul(tmp[:], nm[:], rec[:])
                            nc.vector.tensor_mul(
                                y_bf[b][ih][:, c0 : c0 + 512],
                                tmp[:],
                                sq_sb[b][ih][:, c0 : c0 + 512],
                            )

            # ---- phase F: transpose y, output projection ----
            with (
                tc.tile_pool(name="pf_y", bufs=2) as pf_y,
                tc.tile_pool(name="pf_o", bufs=3) as pf_o,
                tc.tile_pool(name="ps_ft", bufs=2, space="PSUM") as ps_ft,
                tc.tile_pool(name="ps_fo", bufs=2, space="PSUM") as ps_fo,
            ):
                for b in range(B):
                    for ih in range(2):
                        yT = []
                        for dt in range(8):
                            pst = ps_ft.tile([P, P], bf16, tag="pst")
                            nc.tensor.matmul(
                                pst[:],
                                y_bf[b][ih][:, dt * P : (dt + 1) * P],
                                ident[:],
                                is_transpose=True,
                            )
                            yt = pf_y.tile([P, P], bf16, tag=f"yt{dt}", name=f"yt{dt}")
                            nc.scalar.activation(yt[:], pst[:], AF.Copy)
                            yT.append(yt)
                        for es in range(2):
                            pso = ps_fo.tile([P, 512], f32, tag="pso")
                            for dt in range(8):
                                nc.tensor.matmul(
                                    pso[:],
                                    yT[dt][:],
                                    wo_sb[dt][:, es * 512 : (es + 1) * 512],
                                    start=(dt == 0),
                                    stop=(dt == 7),
                                )
                            osb = pf_o.tile([P, 512], f32, tag="osb")
                            nc.scalar.activation(osb[:], pso[:], AF.Copy)
                            nc.sync.dma_start(
                                out=out[b, ih, :, es * 512 : (es + 1) * 512], in_=osb[:]
                            )

    nc.compile()
    return nc


def _prep_inputs(query, key_in, value, Wq, Wk, Wv, pos_bias):
    bf = ml_dtypes.bfloat16
    wq_t = np.ascontiguousarray(Wq.T).astype(bf)
    wk_t = np.ascontiguousarray(Wk.T).astype(bf)
    wv_t = np.ascontiguousarray(Wv.T).astype(bf)

    jj = np.arange(S)
    in_maps = []
    for c in range(NC):
        t0, t1 = c, NT - 1 - c
        i_idx = np.r_[t0 * P : (t0 + 1) * P, t1 * P : (t1 + 1) * P]
        pb = pos_bias[i_idx, :].T.copy()  # [S, 256]
        valid = jj[:, None] <= (i_idx[None, :] - (W - 1))
        pb[~valid] = 0.0

        sl = slice(256 * c, 256 * (c + 1))
        in_maps.append(
            {
                "keyT": np.ascontiguousarray(
                    key_in[sl].transpose(2, 1, 0).reshape(D, 1024)
                ).astype(bf),
                "valT": np.ascontiguousarray(
                    value[sl].transpose(2, 1, 0).reshape(D, 1024)
                ).astype(bf),
                "queryT": np.ascontiguousarray(
                    query[i_idx].transpose(2, 1, 0).reshape(D, 1024)
                ).astype(bf),
                "pbT": pb.astype(bf),
                "wk": wk_t,
                "wv": wv_t,
                "wq": wq_t,
                "wo": None,  # filled by caller (needs Wo)
            }
        )
    return in_maps


def _run(inputs, trace=False):
    from concourse.bass_utils import run_bass_kernel_spmd

    if "nc" not in _CACHE:
        _CACHE["nc"] = _build()
    nc = _CACHE["nc"]

    in_maps = _prep_inputs(
        inputs["query"],
        inputs["key_in"],
        inputs["value"],
        inputs["Wq"],
        inputs["Wk"],
        inputs["Wv"],
        inputs["pos_bias"],
    )
    wo_t = np.ascontiguousarray(inputs["Wo"].T).astype(ml_dtypes.bfloat16)
    for m in in_maps:
        m["wo"] = wo_t

    res = run_bass_kernel_spmd(nc, in_maps, core_ids=list(range(NC)), trace=trace)

    full = np.empty((S, B, D), np.float32)
    for c in range(NC):
        o = np.asarray(res.results[c]["out"], np.float32)  # [B, 2, P, D]
        for ih, t in enumerate((c, NT - 1 - c)):
            full[t * P : (t + 1) * P] = o[:, ih].transpose(1, 0, 2)
    return full, res


def kernel(**inputs):
    inputs = {k: np.asarray(v) for k, v in inputs.items()}
    full, _ = _run(inputs, trace=False)
    return full


if __name__ == "__main__":
    inputs = np.load("/tmp/inputs.npy", allow_pickle=True).item()
    out = kernel(**inputs)
    print("out", out.shape, out.dtype)


# revision 6
# speedup vs baseline: 1.3442x; 1.0232x over previous
"""AFT-Local distributed Trainium2 kernel (8 NeuronCores).

Math (reference, with cancellations):
  q = query @ Wq.T; k = key_in @ Wk.T; v = value @ Wv.T      [S,B,D]
  E[i,j] = exp(pos_bias[i,j] * (j <= i-255))                 [S,S]
  num[i,b,:] = sum_j E[i,j] * (exp(k)*v)[j,b,:]
  den[i,b,:] = sum_j E[i,j] *  exp(k)[j,b,:]
  out = (sigmoid(q) * num / den) @ Wo.T
The max-subtractions in the reference cancel in num/den; all values are small
enough that plain exp is safe.

Distribution: sequence-parallel. Core c owns output row-tiles {c, 15-c}
(128 rows each; paired so every core does identical work). Each core projects
its own 256-row j-shard of k/v, applies exp, and AllGathers bf16 ek / ek*v;
attention + q/o projections are then local. All per-core variation (which
i-rows, masked pos-bias slice) is carried in input data so the SPMD program
is identical on every core.

Perf structure (v2): every matmul phase uses long accumulation chains into a
single PSUM bank (HAM stays warm; LDWEIGHTS overlaps); attention streams
gathered ek/ekv as [128,1024] row-blocks (contiguous 2KB DMA rows); the
num/den epilogue produces y in [i,d]; a PE-transpose turns it into the
stationary operand for the output projection.
"""

import os
import sys

import numpy as np
import ml_dtypes

sys.path.insert(0, "/opt/trn_rl_repo")

S, B, D, W = 2048, 4, 1024, 256
NC = 8
P = 128
NT = S // P  # 16 global i/j tiles

_CACHE = {}


def _build():
    import concourse.bass as bass
    import concourse.bacc as bacc
    import concourse.mybir as mybir
    import concourse.tile as tile
    from concourse.masks import make_identity

    f32 = mybir.dt.float32
    bf16 = mybir.dt.bfloat16
    AF = mybir.ActivationFunctionType

    nc = bacc.Bacc("TRN2", target_bir_lowering=False, debug=False, num_devices=NC)

    keyT = nc.dram_tensor("keyT", [D, 1024], bf16, kind="ExternalInput")
    valT = nc.dram_tensor("valT", [D, 1024], bf16, kind="ExternalInput")
    queryT = nc.dram_tensor("queryT", [D, 1024], bf16, kind="ExternalInput")
    pbT = nc.dram_tensor("pbT", [S, 256], bf16, kind="ExternalInput")
    wk = nc.dram_tensor("wk", [D, D], bf16, kind="ExternalInput")
    wv = nc.dram_tensor("wv", [D, D], bf16, kind="ExternalInput")
    wq = nc.dram_tensor("wq", [D, D], bf16, kind="ExternalInput")
    wo = nc.dram_tensor("wo", [D, D], bf16, kind="ExternalInput")
    out = nc.dram_tensor("out", [B, 2, P, D], f32, kind="ExternalOutput")

    with tile.TileContext(nc) as tc:
        with (
            tc.tile_pool(name="persist", bufs=1) as persist,
            tc.tile_pool(name="dram", bufs=1, space="DRAM") as dram,
        ):
            ag_in_a = dram.tile([512, 2048], bf16)
            ag_in_b = dram.tile([512, 2048], bf16)
            ag_out_a = dram.tile([NC * 512, 2048], bf16, addr_space="Shared")
            ag_out_b = dram.tile([NC * 512, 2048], bf16, addr_space="Shared")

            # ---- phase A: k/v projection on own j-shard, exp, stage for AG ----
            with (
                tc.tile_pool(name="pa", bufs=1) as pa,
                tc.tile_pool(name="pa_st", bufs=3) as pa_st,
                tc.tile_pool(name="ps_a", bufs=2, space="PSUM") as ps_a,
            ):
                keyT_sb = []
                valT_sb = []
                wk_sb = []
                wv_sb = []
                for kt in range(8):
                    a1 = pa.tile([P, 1024], bf16, name=f"keyT{kt}")
                    nc.sync.dma_start(out=a1[:], in_=keyT[kt * P : (kt + 1) * P, :])
                    keyT_sb.append(a1)
                    a2 = pa.tile([P, 1024], bf16, name=f"valT{kt}")
                    nc.sync.dma_start(out=a2[:], in_=valT[kt * P : (kt + 1) * P, :])
                    valT_sb.append(a2)
                    a3 = pa.tile([P, D], bf16, name=f"wk{kt}")
                    nc.sync.dma_start(out=a3[:], in_=wk[kt * P : (kt + 1) * P, :])
                    wk_sb.append(a3)
                    a4 = pa.tile([P, D], bf16, name=f"wv{kt}")
                    nc.sync.dma_start(out=a4[:], in_=wv[kt * P : (kt + 1) * P, :])
                    wv_sb.append(a4)

                for tt in range(8):
                    ag_in = ag_in_a if tt < 4 else ag_in_b
                    rt = (tt % 4) * P
                    for es in range(2):
                        psk = ps_a.tile([P, 512], f32, tag="psk")
                        psv = ps_a.tile([P, 512], f32, tag="psv")
                        for kt in range(8):
                            nc.tensor.matmul(
                                psk[:],
                                keyT_sb[kt][:, tt * P : (tt + 1) * P],
                                wk_sb[kt][:, es * 512 : (es + 1) * 512],
                                start=(kt == 0),
                                stop=(kt == 7),
                            )
                        for kt in range(8):
                            nc.tensor.matmul(
                                psv[:],
                                valT_sb[kt][:, tt * P : (tt + 1) * P],
                                wv_sb[kt][:, es * 512 : (es + 1) * 512],
                                start=(kt == 0),
                                stop=(kt == 7),
                            )
                        ekf = pa_st.tile([P, 512], f32, tag="ekf")
                        nc.scalar.activation(ekf[:], psk[:], AF.Exp)
                        ekb = pa_st.tile([P, 512], bf16, tag="ekb")
                        nc.vector.tensor_copy(ekb[:], ekf[:])
                        ekvb = pa_st.tile([P, 512], bf16, tag="ekvb")
                        nc.vector.tensor_mul(ekvb[:], ekf[:], psv[:])
                        nc.sync.dma_start(
                            out=ag_in[rt : rt + P, es * 512 : (es + 1) * 512],
                            in_=ekb[:],
                        )
                        nc.sync.dma_start(
                            out=ag_in[
                                rt : rt + P, 1024 + es * 512 : 1024 + (es + 1) * 512
                            ],
                            in_=ekvb[:],
                        )
                    if tt == 3:
                        # first token-half (b 0,1) staged -> gather it while
                        # the second half is still being projected
                        nc.gpsimd.collective_compute(
                            "AllGather",
                            mybir.AluOpType.bypass,
                            ins=[ag_in_a.opt()],
                            outs=[ag_out_a.opt()],
                            replica_groups=[list(range(NC))],
                        )

            nc.gpsimd.collective_compute(
                "AllGather",
                mybir.AluOpType.bypass,
                ins=[ag_in_b.opt()],
                outs=[ag_out_b.opt()],
                replica_groups=[list(range(NC))],
            )

            # ---- persistent tensors needed from phase C on ----
            wq_sb = []
            qT_sb = []
            wo_sb = []
            for kt in range(8):
                t1 = persist.tile([P, D], bf16, name=f"wq{kt}")
                nc.sync.dma_start(out=t1[:], in_=wq[kt * P : (kt + 1) * P, :])
                wq_sb.append(t1)
                t2 = persist.tile([P, 1024], bf16, name=f"qT{kt}")
                nc.sync.dma_start(out=t2[:], in_=queryT[kt * P : (kt + 1) * P, :])
                qT_sb.append(t2)
                t3 = persist.tile([P, D], bf16, name=f"wo{kt}")
                nc.sync.dma_start(out=t3[:], in_=wo[kt * P : (kt + 1) * P, :])
                wo_sb.append(t3)
            e_sb = [persist.tile([P, 256], bf16, name=f"e{jt}") for jt in range(NT)]
            sq_sb = [
                [persist.tile([P, 1024], bf16, name=f"sq{b}_{ih}") for ih in range(2)]
                for b in range(B)
            ]
            y_bf = [
                [persist.tile([P, 1024], bf16, name=f"y{b}_{ih}") for ih in range(2)]
                for b in range(B)
            ]
            ident = persist.tile([P, P], bf16, name="ident")
            make_identity(nc, ident[:])

            # ---- phase D: E = exp(host-masked pos_bias^T slice) ----
            with tc.tile_pool(name="pd", bufs=3) as pd:
                for jt in range(NT):
                    pbt = pd.tile([P, 256], bf16, tag="pbt")
                    nc.sync.dma_start(out=pbt[:], in_=pbT[jt * P : (jt + 1) * P, :])
                    nc.scalar.activation(e_sb[jt][:], pbt[:], AF.Exp)

            # ---- phase C: q projection + sigmoid (overlaps the AllGather) ----
            with tc.tile_pool(name="ps_c", bufs=2, space="PSUM") as ps_c:
                for b in range(B):
                    for ih in range(2):
                        it = b * 2 + ih
                        psqa = ps_c.tile([P, 512], f32, tag="psqa")
                        psqb = ps_c.tile([P, 512], f32, tag="psqb")
                        for kt in range(8):
                            nc.tensor.matmul(
                                psqa[:],
                                qT_sb[kt][:, it * P : (it + 1) * P],
                                wq_sb[kt][:, 0:512],
                                start=(kt == 0),
                                stop=(kt == 7),
                            )
                        for kt in range(8):
                            nc.tensor.matmul(
                                psqb[:],
                                qT_sb[kt][:, it * P : (it + 1) * P],
                                wq_sb[kt][:, 512:1024],
                                start=(kt == 0),
                                stop=(kt == 7),
                            )
                        nc.scalar.activation(sq_sb[b][ih][:, 0:512], psqa[:], AF.Sigmoid)
                        nc.scalar.activation(
                            sq_sb[b][ih][:, 512:1024], psqb[:], AF.Sigmoid
                        )

            # ---- phase E: num/den chains over gathered ek/ekv; y epilogue ----
            with (
                tc.tile_pool(name="pe_blk", bufs=1) as pe_blk,
                tc.tile_pool(name="pe_ep", bufs=2) as pe_ep,
                tc.tile_pool(name="ps_e", bufs=2, space="PSUM") as ps_e,
            ):
                for b in range(B):
                    ag_out = ag_out_a if b < 2 else ag_out_b
                    sub = b % 2
                    # four half-blocks per jt; each feeds exactly one chain per
                    # ih, ordered so a buffer's last read lands ~3 chains
                    # before the next b needs it -> DMA fully pipelined.
                    blks = {}
                    for kind, c0 in (("ekvlo", 1024), ("ekvhi", 1536), ("eklo", 0), ("ekhi", 512)):
                        for jt in range(NT):
                            r0 = 512 * (jt // 2) + 256 * sub + P * (jt % 2)
                            t = pe_blk.tile(
                                [P, 512], bf16, tag=f"{kind}{jt}", name=f"{kind}{jt}"
                            )
                            nc.sync.dma_start(
                                out=t[:], in_=ag_out[r0 : r0 + P, c0 : c0 + 512]
                            )
                            blks[(kind, jt)] = t
                    for ih in range(2):
                        isl = slice(ih * P, (ih + 1) * P)
                        na = ps_e.tile([P, 512], f32, tag="na")
                        nb = ps_e.tile([P, 512], f32, tag="nb")
                        da = ps_e.tile([P, 512], f32, tag="da")
                        db = ps_e.tile([P, 512], f32, tag="db")
                        for tgt, kind in (
                            (na, "ekvlo"),
                            (nb, "ekvhi"),
                            (da, "eklo"),
                            (db, "ekhi"),
                        ):
                            for jt in range(NT):
                                nc.tensor.matmul(
                                    tgt[:],
                                    e_sb[jt][:, isl],
                                    blks[(kind, jt)][:, :],
                                    start=(jt == 0),
                                    stop=(jt == NT - 1),
                                )
                        for half, (nm, dn) in enumerate(((na, da), (nb, db))):
                            c0 = half * 512
                            rec = pe_ep.tile([P, 512], f32, tag="rec")
                            nc.vector.reciprocal(rec[:], dn[:])
                            tmp = pe_ep.tile([P, 512], f32, tag="tmp")
                            nc.vector.tensor_mul(tmp[:], nm[:], rec[:])
                            nc.vector.tensor_mul(
                                y_bf[b][ih][:, c0 : c0 + 512],
                                tmp[:],
                                sq_sb[b][ih][:, c0 : c0 + 512],
                            )

            # ---- phase F: transpose y, output projection ----
            with (
                tc.tile_pool(name="pf_y", bufs=2) as pf_y,
                tc.tile_pool(name="pf_o", bufs=3) as pf_o,
                tc.tile_pool(name="ps_ft", bufs=2, space="PSUM") as ps_ft,
                tc.tile_pool(name="ps_fo", bufs=2, space="PSUM") as ps_fo,
            ):
                for b in range(B):
                    for ih in range(2):
                        yT = []
                        for dt in range(8):
                            pst = ps_ft.tile([P, P], bf16, tag="pst")
                            nc.tensor.matmul(
                                pst[:],
                                y_bf[b][ih][:, dt * P : (dt + 1) * P],
                                ident[:],
                                is_transpose=True,
                            )
                            yt = pf_y.tile([P, P], bf16, tag=f"yt{dt}", name=f"yt{dt}")
                            nc.scalar.activation(yt[:], pst[:], AF.Copy)
                            yT.append(yt)
                        for es in range(2):
                            pso = ps_fo.tile([P, 512], f32, tag="pso")
                            for dt in range(8):
                                nc.tensor.matmul(
                                    pso[:],
                                    yT[dt][:],
                                    wo_sb[dt][:, es * 512 : (es + 1) * 512],
                                    start=(dt == 0),
                                    stop=(dt == 7),
                                )
                            osb = pf_o.tile([P, 512], f32, tag="osb")
                            nc.scalar.activation(osb[:], pso[:], AF.Copy)
                            nc.sync.dma_start(
                                out=out[b, ih, :, es * 512 : (es + 1) * 512], in_=osb[:]
                            )

    nc.compile()
    return nc


def _prep_inputs(query, key_in, value, Wq, Wk, Wv, pos_bias):
    bf = ml_dtypes.bfloat16
    wq_t = np.ascontiguousarray(Wq.T).astype(bf)
    wk_t = np.ascontiguousarray(Wk.T).astype(bf)
    wv_t = np.ascontiguousarray(Wv.T).astype(bf)

    jj = np.arange(S)
    in_maps = []
    for c in range(NC):
        t0, t1 = c, NT - 1 - c
        i_idx = np.r_[t0 * P : (t0 + 1) * P, t1 * P : (t1 + 1) * P]
        pb = pos_bias[i_idx, :].T.copy()  # [S, 256]
        valid = jj[:, None] <= (i_idx[None, :] - (W - 1))
        pb[~valid] = 0.0

        sl = slice(256 * c, 256 * (c + 1))
        in_maps.append(
            {
                "keyT": np.ascontiguousarray(
                    key_in[sl].transpose(2, 1, 0).reshape(D, 1024)
                ).astype(bf),
                "valT": np.ascontiguousarray(
                    value[sl].transpose(2, 1, 0).reshape(D, 1024)
                ).astype(bf),
                "queryT": np.ascontiguousarray(
                    query[i_idx].transpose(2, 1, 0).reshape(D, 1024)
                ).astype(bf),
                "pbT": pb.astype(bf),
                "wk": wk_t,
                "wv": wv_t,
                "wq": wq_t,
                "wo": None,  # filled by caller (needs Wo)
            }
        )
    return in_maps


def _run(inputs, trace=False):
    from concourse.bass_utils import run_bass_kernel_spmd

    if "nc" not in _CACHE:
        _CACHE["nc"] = _build()
    nc = _CACHE["nc"]

    in_maps = _prep_inputs(
        inputs["query"],
        inputs["key_in"],
        inputs["value"],
        inputs["Wq"],
        inputs["Wk"],
        inputs["Wv"],
        inputs["pos_bias"],
    )
    wo_t = np.ascontiguousarray(inputs["Wo"].T).astype(ml_dtypes.bfloat16)
    for m in in_maps:
        m["wo"] = wo_t

    res = run_bass_kernel_spmd(nc, in_maps, core_ids=list(range(NC)), trace=trace)

    full = np.empty((S, B, D), np.float32)
    for c in range(NC):
        o = np.asarray(res.results[c]["out"], np.float32)  # [B, 2, P, D]
        for ih, t in enumerate((c, NT - 1 - c)):
            full[t * P : (t + 1) * P] = o[:, ih].transpose(1, 0, 2)
    return full, res


def kernel(**inputs):
    inputs = {k: np.asarray(v) for k, v in inputs.items()}
    full, _ = _run(inputs, trace=False)
    return full


if __name__ == "__main__":
    inputs = np.load("/tmp/inputs.npy", allow_pickle=True).item()
    out = kernel(**inputs)
    print("out", out.shape, out.dtype)


# revision 9
# speedup vs baseline: 1.6264x; 1.2099x over previous
"""AFT-Local distributed Trainium2 kernel (8 NeuronCores).

Math (reference, with cancellations):
  q = query @ Wq.T; k = key_in @ Wk.T; v = value @ Wv.T      [S,B,D]
  E[i,j] = exp(pos_bias[i,j] * (j <= i-255))                 [S,S]
  num[i,b,:] = sum_j E[i,j] * (exp(k)*v)[j,b,:]
  den[i,b,:] = sum_j E[i,j] *  exp(k)[j,b,:]
  out = (sigmoid(q) * num / den) @ Wo.T
The max-subtractions in the reference cancel in num/den; all values are small
enough that plain exp is safe.

Distribution (v4): pure data/tensor-parallel, ZERO device collectives (the
collective control path on this fleet has a ~90us fixed cost, impossible to
hide). Core c owns (batch b = c//2, d-half h = c%2): it projects k/v/q for
all 2048 tokens restricted to its 512 d-columns (no duplicated FLOPs),
runs the full [2048x2048] E-weighted attention on its slice entirely out of
SBUF, and computes a PARTIAL output projection over its d-half. The host
sums each core-pair's f32 partials while unsharding - the only cross-core
data motion in the whole scheme.

All matmuls are bf16 with f32 PSUM accumulation, structured as 16-deep
accumulation chains into a single PSUM bank (keeps the PE HAM-warm). The
local mask is pre-applied to pos_bias^T on the host (static index mask) so
the device only exponentiates.
"""

import os
import sys

import numpy as np
import ml_dtypes

sys.path.insert(0, "/opt/trn_rl_repo")

S, B, D, W = 2048, 4, 1024, 256
NC = 8
P = 128
NT = S // P  # 16 token/row tiles
DH = 512  # d-half owned per core

_CACHE = {}


def _build():
    import concourse.bass as bass
    import concourse.bacc as bacc
    import concourse.mybir as mybir
    import concourse.tile as tile
    from concourse.masks import make_identity

    f32 = mybir.dt.float32
    bf16 = mybir.dt.bfloat16
    AF = mybir.ActivationFunctionType

    nc = bacc.Bacc("TRN2", target_bir_lowering=False, debug=False, num_devices=NC)

    # per-core inputs (b = batch owned, h = d-half owned)
    keyT = nc.dram_tensor("keyT", [D, S], bf16, kind="ExternalInput")  # key_in[:,b,:].T
    valT = nc.dram_tensor("valT", [D, S], bf16, kind="ExternalInput")
    queryT = nc.dram_tensor("queryT", [D, S], bf16, kind="ExternalInput")
    pbT = nc.dram_tensor("pbT", [S, S], bf16, kind="ExternalInput")  # masked pos_bias^T
    wk = nc.dram_tensor("wk", [D, DH], bf16, kind="ExternalInput")  # Wk.T[:, h-cols]
    wv = nc.dram_tensor("wv", [D, DH], bf16, kind="ExternalInput")
    wq = nc.dram_tensor("wq", [D, DH], bf16, kind="ExternalInput")
    wo = nc.dram_tensor("wo", [DH, D], bf16, kind="ExternalInput")  # Wo.T[h-rows, :]
    out = nc.dram_tensor("out", [S, D], f32, kind="ExternalOutput")  # partial!

    with tile.TileContext(nc) as tc:
        with tc.tile_pool(name="persist", bufs=1) as persist:
            # resident across phases
            ek_sb = [persist.tile([P, DH], bf16, name=f"ek{t}") for t in range(NT)]
            ekv_sb = [persist.tile([P, DH], bf16, name=f"ekv{t}") for t in range(NT)]
            eT_sb = [persist.tile([P, S], bf16, name=f"eT{t}") for t in range(NT)]
            sq_sb = [persist.tile([P, DH], bf16, name=f"sq{t}") for t in range(NT)]
            y_sb = [persist.tile([P, DH], bf16, name=f"y{t}") for t in range(NT)]
            ident = persist.tile([P, P], bf16, name="ident")
            make_identity(nc, ident[:])

            # ---- phase A: k/v projection (all tokens, own d-half), exp ----
            # two token-halves so keyT/valT are only half-resident
            with (
                tc.tile_pool(name="pa", bufs=1) as pa,
                tc.tile_pool(name="pa_st", bufs=3) as pa_st,
                tc.tile_pool(name="ps_a", bufs=2, space="PSUM") as ps_a,
            ):
                wk_sb = []
                wv_sb = []
                for kt in range(8):
                    a3 = pa.tile([P, DH], bf16, name=f"wk{kt}")
                    nc.sync.dma_start(out=a3[:], in_=wk[kt * P : (kt + 1) * P, :])
                    wk_sb.append(a3)
                    a4 = pa.tile([P, DH], bf16, name=f"wv{kt}")
                    nc.sync.dma_start(out=a4[:], in_=wv[kt * P : (kt + 1) * P, :])
                    wv_sb.append(a4)
                for half in range(2):
                    cs = slice(half * 1024, (half + 1) * 1024)
                    keyT_sb = []
                    valT_sb = []
                    for kt in range(8):
                        a1 = pa.tile([P, 1024], bf16, tag=f"keyT{kt}", name=f"keyT{kt}")
                        nc.sync.dma_start(out=a1[:], in_=keyT[kt * P : (kt + 1) * P, cs])
                        keyT_sb.append(a1)
                        a2 = pa.tile([P, 1024], bf16, tag=f"valT{kt}", name=f"valT{kt}")
                        nc.sync.dma_start(out=a2[:], in_=valT[kt * P : (kt + 1) * P, cs])
                        valT_sb.append(a2)
                    for tl in range(8):
                        tt = half * 8 + tl
                        psk = ps_a.tile([P, DH], f32, tag="psk")
                        psv = ps_a.tile([P, DH], f32, tag="psv")
                        for kt in range(8):
                            nc.tensor.matmul(
                                psk[:],
                                keyT_sb[kt][:, tl * P : (tl + 1) * P],
                                wk_sb[kt][:],
                                start=(kt == 0),
                                stop=(kt == 7),
                            )
                        for kt in range(8):
                            nc.tensor.matmul(
                                psv[:],
                                valT_sb[kt][:, tl * P : (tl + 1) * P],
                                wv_sb[kt][:],
                                start=(kt == 0),
                                stop=(kt == 7),
                            )
                        ekf = pa_st.tile([P, DH], f32, tag="ekf")
                        nc.scalar.activation(ekf[:], psk[:], AF.Exp)
                        nc.vector.tensor_copy(ek_sb[tt][:], ekf[:])
                        nc.vector.tensor_mul(ekv_sb[tt][:], ekf[:], psv[:])

            # ---- phase C: q projection + sigmoid; D: E = exp(pbT) on ACT ----
            with (
                tc.tile_pool(name="pc", bufs=1) as pc,
                tc.tile_pool(name="pd", bufs=3) as pd,
                tc.tile_pool(name="ps_c", bufs=2, space="PSUM") as ps_c,
            ):
                qT_sb = []
                wq_sb = []
                for kt in range(8):
                    c2 = pc.tile([P, DH], bf16, name=f"wq{kt}")
                    nc.sync.dma_start(out=c2[:], in_=wq[kt * P : (kt + 1) * P, :])
                    wq_sb.append(c2)
                    c1 = pc.tile([P, S], bf16, name=f"qT{kt}")
                    nc.sync.dma_start(out=c1[:], in_=queryT[kt * P : (kt + 1) * P, :])
                    qT_sb.append(c1)
                for jt in range(NT):
                    pbt = pd.tile([P, S], bf16, tag="pbt")
                    nc.sync.dma_start(out=pbt[:], in_=pbT[jt * P : (jt + 1) * P, :])
                    nc.scalar.activation(eT_sb[jt][:], pbt[:], AF.Exp)
                for it in range(NT):
                    psq = ps_c.tile([P, DH], f32, tag="psq")
                    for kt in range(8):
                        nc.tensor.matmul(
                            psq[:],
                            qT_sb[kt][:, it * P : (it + 1) * P],
                            wq_sb[kt][:],
                            start=(kt == 0),
                            stop=(kt == 7),
                        )
                    nc.scalar.activation(sq_sb[it][:], psq[:], AF.Sigmoid)

            # ---- phase E: num/den 16-chains; y epilogue ----
            with (
                tc.tile_pool(name="pe_ep", bufs=2) as pe_ep,
                tc.tile_pool(name="ps_e", bufs=2, space="PSUM") as ps_e,
            ):
                for it in range(NT):
                    isl = slice(it * P, (it + 1) * P)
                    na = ps_e.tile([P, DH], f32, tag="na")
                    da = ps_e.tile([P, DH], f32, tag="da")
                    for jt in range(NT):
                        nc.tensor.matmul(
                            na[:],
                            eT_sb[jt][:, isl],
                            ekv_sb[jt][:],
                            start=(jt == 0),
                            stop=(jt == NT - 1),
                        )
                    for jt in range(NT):
                        nc.tensor.matmul(
                            da[:],
                            eT_sb[jt][:, isl],
                            ek_sb[jt][:],
                            start=(jt == 0),
                            stop=(jt == NT - 1),
                        )
                    rec = pe_ep.tile([P, DH], f32, tag="rec")
                    nc.vector.reciprocal(rec[:], da[:])
                    tmp = pe_ep.tile([P, DH], f32, tag="tmp")
                    nc.vector.tensor_mul(tmp[:], na[:], rec[:])
                    nc.vector.tensor_mul(y_sb[it][:], tmp[:], sq_sb[it][:])

            # ---- phase F: transpose y, partial output projection ----
            with (
                tc.tile_pool(name="pf_y", bufs=2) as pf_y,
                tc.tile_pool(name="pf_o", bufs=3) as pf_o,
                tc.tile_pool(name="ps_ft", bufs=2, space="PSUM") as ps_ft,
                tc.tile_pool(name="ps_fo", bufs=2, space="PSUM") as ps_fo,
            ):
                wo_sb = []
                for dt in range(4):
                    w1 = pf_o.tile([P, D], bf16, name=f"wo{dt}", tag=f"wo{dt}", bufs=1)
                    nc.sync.dma_start(out=w1[:], in_=wo[dt * P : (dt + 1) * P, :])
                    wo_sb.append(w1)
                for it in range(NT):
                    yT = []
                    for dt in range(4):
                        pst = ps_ft.tile([P, P], bf16, tag="pst")
                        nc.tensor.matmul(
                            pst[:],
                            y_sb[it][:, dt * P : (dt + 1) * P],
                            ident[:],
                            is_transpose=True,
                        )
                        yt = pf_y.tile([P, P], bf16, tag=f"yt{dt}", name=f"yt{dt}")
                        nc.scalar.activation(yt[:], pst[:], AF.Copy)
                        yT.append(yt)
                    for es in range(2):
                        pso = ps_fo.tile([P, 512], f32, tag="pso")
                        for dt in range(4):
                            nc.tensor.matmul(
                                pso[:],
                                yT[dt][:],
                                wo_sb[dt][:, es * 512 : (es + 1) * 512],
                                start=(dt == 0),
                                stop=(dt == 3),
                            )
                        osb = pf_o.tile([P, 512], f32, tag="osb")
                        nc.scalar.activation(osb[:], pso[:], AF.Copy)
                        nc.sync.dma_start(
                            out=out[it * P : (it + 1) * P, es * 512 : (es + 1) * 512],
                            in_=osb[:],
                        )

    nc.compile()
    return nc


def _prep_inputs(inputs):
    bf = ml_dtypes.bfloat16
    query, key_in, value = inputs["query"], inputs["key_in"], inputs["value"]
    pos_bias = inputs["pos_bias"]

    jj = np.arange(S)
    pbT = pos_bias.T.copy()  # [j, i]
    pbT[~(jj[:, None] <= jj[None, :] - (W - 1))] = 0.0
    pbT = pbT.astype(bf)

    wq_t = np.ascontiguousarray(inputs["Wq"].T).astype(bf)  # [din, e]
    wk_t = np.ascontiguousarray(inputs["Wk"].T).astype(bf)
    wv_t = np.ascontiguousarray(inputs["Wv"].T).astype(bf)
    wo_t = np.ascontiguousarray(inputs["Wo"].T).astype(bf)  # [d, e']

    keyT_b = [np.ascontiguousarray(key_in[:, b, :].T).astype(bf) for b in range(B)]
    valT_b = [np.ascontiguousarray(value[:, b, :].T).astype(bf) for b in range(B)]
    qT_b = [np.ascontiguousarray(query[:, b, :].T).astype(bf) for b in range(B)]

    in_maps = []
    for c in range(NC):
        b, h = c // 2, c % 2
        hs = slice(h * DH, (h + 1) * DH)
        in_maps.append(
            {
                "keyT": keyT_b[b],
                "valT": valT_b[b],
                "queryT": qT_b[b],
                "pbT": pbT,
                "wk": np.ascontiguousarray(wk_t[:, hs]),
                "wv": np.ascontiguousarray(wv_t[:, hs]),
                "wq": np.ascontiguousarray(wq_t[:, hs]),
                "wo": np.ascontiguousarray(wo_t[hs, :]),
            }
        )
    return in_maps


def _run(inputs, trace=False):
    from concourse.bass_utils import run_bass_kernel_spmd

    if "nc" not in _CACHE:
        _CACHE["nc"] = _build()
    nc = _CACHE["nc"]

    in_maps = _prep_inputs(inputs)
    res = run_bass_kernel_spmd(nc, in_maps, core_ids=list(range(NC)), trace=trace)

    # unshard: partial sums over d-halves per batch
    full = np.empty((S, B, D), np.float32)
    for b in range(B):
        p0 = np.asarray(res.results[2 * b]["out"], np.float32)
        p1 = np.asarray(res.results[2 * b + 1]["out"], np.float32)
        full[:, b, :] = p0 + p1
    return full, res


def kernel(**inputs):
    inputs = {k: np.asarray(v) for k, v in inputs.items()}
    full, _ = _run(inputs, trace=False)
    return full


if __name__ == "__main__":
    inputs = np.load("/tmp/inputs.npy", allow_pickle=True).item()
    out = kernel(**inputs)
    print("out", out.shape, out.dtype)


# revision 10
# speedup vs baseline: 1.9467x; 1.1969x over previous
"""AFT-Local distributed Trainium2 kernel (8 NeuronCores).

Math (reference, with cancellations):
  q = query @ Wq.T; k = key_in @ Wk.T; v = value @ Wv.T      [S,B,D]
  E[i,j] = exp(pos_bias[i,j] * (j <= i-255))                 [S,S]
  num[i,b,:] = sum_j E[i,j] * (exp(k)*v)[j,b,:]
  den[i,b,:] = sum_j E[i,j] *  exp(k)[j,b,:]
  out = (sigmoid(q) * num / den) @ Wo.T
The max-subtractions in the reference cancel in num/den; all values are small
enough that plain exp is safe.

Distribution (v4+): pure data/tensor-parallel, ZERO device collectives (the
collective control path on this fleet has a ~90us fixed cost, impossible to
hide). Core c owns (batch b = c//2, d-half h = c%2): it projects k/v/q for
all 2048 tokens restricted to its 512 d-columns (no duplicated FLOPs), runs
the full [2048x2048] E-weighted attention on its slice entirely out of SBUF,
and computes a PARTIAL output projection over its d-half. The host sums each
core-pair's f32 partials while unsharding - the only cross-core data motion
in the whole scheme.

Kernel structure (v5): all matmuls bf16 with f32 PSUM accumulation, in long
accumulation chains into a single PSUM bank (keeps the PE HAM-warm). The
attention num/den and the q projection run in the TRANSPOSED [d,i]
orientation so y comes out as y^T and feeds the output projection directly -
no on-chip transposes anywhere. The local mask is pre-applied to pos_bias^T
on the host (static index mask) so the device only exponentiates.
"""

import os
import sys

import numpy as np
import ml_dtypes

sys.path.insert(0, "/opt/trn_rl_repo")

S, B, D, W = 2048, 4, 1024, 256
NC = 8
P = 128
NT = S // P  # 16 token/row tiles
DH = 512  # d-half owned per core

_CACHE = {}


def _build():
    import concourse.bass as bass
    import concourse.bacc as bacc
    import concourse.mybir as mybir
    import concourse.tile as tile

    f32 = mybir.dt.float32
    bf16 = mybir.dt.bfloat16
    AF = mybir.ActivationFunctionType

    nc = bacc.Bacc("TRN2", target_bir_lowering=False, debug=False, num_devices=NC)

    # per-core inputs (b = batch owned, h = d-half owned)
    keyT = nc.dram_tensor("keyT", [D, S], bf16, kind="ExternalInput")  # key_in[:,b,:].T
    valT = nc.dram_tensor("valT", [D, S], bf16, kind="ExternalInput")
    queryT = nc.dram_tensor("queryT", [D, S], bf16, kind="ExternalInput")
    pbT = nc.dram_tensor("pbT", [S, S], bf16, kind="ExternalInput")  # masked pos_bias^T
    wk = nc.dram_tensor("wk", [D, DH], bf16, kind="ExternalInput")  # Wk.T[:, h-cols]
    wv = nc.dram_tensor("wv", [D, DH], bf16, kind="ExternalInput")
    wq = nc.dram_tensor("wq", [D, DH], bf16, kind="ExternalInput")
    wo = nc.dram_tensor("wo", [DH, D], bf16, kind="ExternalInput")  # Wo.T[h-rows, :]
    out = nc.dram_tensor("out", [S, D], f32, kind="ExternalOutput")  # partial!

    with tile.TileContext(nc) as tc:
        with tc.tile_pool(name="persist", bufs=1) as persist:
            # resident across phases (per-partition KB in comments)
            ek_sb = [persist.tile([P, DH], bf16, name=f"ek{t}") for t in range(NT)]    # 16
            ekv_sb = [persist.tile([P, DH], bf16, name=f"ekv{t}") for t in range(NT)]  # 16
            eT_sb = [persist.tile([P, S], bf16, name=f"eT{t}") for t in range(NT)]     # 64
            sqT_sb = [persist.tile([P, S], bf16, name=f"sqT{t}") for t in range(4)]    # 16
            yT_sb = [persist.tile([P, S], bf16, name=f"yT{t}") for t in range(4)]      # 16

            # ---- phase A: k/v projection (all tokens, own d-half), exp ----
            # two token-halves so keyT/valT are only half-resident
            with (
                tc.tile_pool(name="pa", bufs=1) as pa,
                tc.tile_pool(name="pa_st", bufs=3) as pa_st,
                tc.tile_pool(name="ps_a", bufs=2, space="PSUM") as ps_a,
            ):
                wk_sb = []
                wv_sb = []
                for kt in range(8):
                    a3 = pa.tile([P, DH], bf16, name=f"wk{kt}")
                    nc.sync.dma_start(out=a3[:], in_=wk[kt * P : (kt + 1) * P, :])
                    wk_sb.append(a3)
                    a4 = pa.tile([P, DH], bf16, name=f"wv{kt}")
                    nc.sync.dma_start(out=a4[:], in_=wv[kt * P : (kt + 1) * P, :])
                    wv_sb.append(a4)
                for half in range(2):
                    cs = slice(half * 1024, (half + 1) * 1024)
                    keyT_sb = []
                    valT_sb = []
                    for kt in range(8):
                        a1 = pa.tile([P, 1024], bf16, tag=f"keyT{kt}", name=f"keyT{kt}")
                        nc.sync.dma_start(out=a1[:], in_=keyT[kt * P : (kt + 1) * P, cs])
                        keyT_sb.append(a1)
                        a2 = pa.tile([P, 1024], bf16, tag=f"valT{kt}", name=f"valT{kt}")
                        nc.sync.dma_start(out=a2[:], in_=valT[kt * P : (kt + 1) * P, cs])
                        valT_sb.append(a2)
                    for tl in range(8):
                        tt = half * 8 + tl
                        psk = ps_a.tile([P, DH], f32, tag="psk")
                        psv = ps_a.tile([P, DH], f32, tag="psv")
                        for kt in range(8):
                            nc.tensor.matmul(
                                psk[:],
                                keyT_sb[kt][:, tl * P : (tl + 1) * P],
                                wk_sb[kt][:],
                                start=(kt == 0),
                                stop=(kt == 7),
                            )
                        for kt in range(8):
                            nc.tensor.matmul(
                                psv[:],
                                valT_sb[kt][:, tl * P : (tl + 1) * P],
                                wv_sb[kt][:],
                                start=(kt == 0),
                                stop=(kt == 7),
                            )
                        ekf = pa_st.tile([P, DH], f32, tag="ekf")
                        nc.scalar.activation(ekf[:], psk[:], AF.Exp)
                        nc.vector.tensor_copy(ek_sb[tt][:], ekf[:])
                        nc.vector.tensor_mul(ekv_sb[tt][:], ekf[:], psv[:])

            # ---- phase D: E = exp(pbT); loads + ACT exp overlap A/C compute ----
            with tc.tile_pool(name="pd", bufs=3) as pd:
                for jt in range(NT):
                    pbt = pd.tile([P, S], bf16, tag="pbt")
                    nc.sync.dma_start(out=pbt[:], in_=pbT[jt * P : (jt + 1) * P, :])
                    nc.scalar.activation(eT_sb[jt][:], pbt[:], AF.Exp)

            # ---- phase C: q^T projection + sigmoid ([e,i] orientation) ----
            with (
                tc.tile_pool(name="pc", bufs=1) as pc,
                tc.tile_pool(name="ps_c", bufs=2, space="PSUM") as ps_c,
            ):
                qT_sb = []
                wq_sb = []
                for kt in range(8):
                    c2 = pc.tile([P, DH], bf16, name=f"wq{kt}")
                    nc.sync.dma_start(out=c2[:], in_=wq[kt * P : (kt + 1) * P, :])
                    wq_sb.append(c2)
                    c1 = pc.tile([P, S], bf16, name=f"qT{kt}")
                    nc.sync.dma_start(out=c1[:], in_=queryT[kt * P : (kt + 1) * P, :])
                    qT_sb.append(c1)
                for et in range(4):
                    for ib in range(4):
                        psq = ps_c.tile([P, 512], f32, tag="psq")
                        for kt in range(8):
                            nc.tensor.matmul(
                                psq[:],
                                wq_sb[kt][:, et * P : (et + 1) * P],
                                qT_sb[kt][:, ib * 512 : (ib + 1) * 512],
                                start=(kt == 0),
                                stop=(kt == 7),
                            )
                        nc.scalar.activation(
                            sqT_sb[et][:, ib * 512 : (ib + 1) * 512], psq[:], AF.Sigmoid
                        )

            # ---- phase E: num^T/den^T 16-chains in [d,i]; y^T epilogue ----
            with (
                tc.tile_pool(name="pe_ep", bufs=2) as pe_ep,
                tc.tile_pool(name="ps_e", bufs=2, space="PSUM") as ps_e,
            ):
                for ib in range(4):
                    csl = slice(ib * 512, (ib + 1) * 512)
                    for dt in range(4):
                        dsl = slice(dt * P, (dt + 1) * P)
                        na = ps_e.tile([P, 512], f32, tag="na")
                        da = ps_e.tile([P, 512], f32, tag="da")
                        for jt in range(NT):
                            nc.tensor.matmul(
                                na[:],
                                ekv_sb[jt][:, dsl],
                                eT_sb[jt][:, csl],
                                start=(jt == 0),
                                stop=(jt == NT - 1),
                            )
                        for jt in range(NT):
                            nc.tensor.matmul(
                                da[:],
                                ek_sb[jt][:, dsl],
                                eT_sb[jt][:, csl],
                                start=(jt == 0),
                                stop=(jt == NT - 1),
                            )
                        rec = pe_ep.tile([P, 512], f32, tag="rec")
                        nc.vector.reciprocal(rec[:], da[:])
                        tmp = pe_ep.tile([P, 512], f32, tag="tmp")
                        nc.vector.tensor_mul(tmp[:], na[:], rec[:])
                        nc.vector.tensor_mul(
                            yT_sb[dt][:, csl], tmp[:], sqT_sb[dt][:, csl]
                        )

            # ---- phase F: partial output projection (y^T is the lhsT) ----
            with (
                tc.tile_pool(name="pf_o", bufs=3) as pf_o,
                tc.tile_pool(name="ps_fo", bufs=2, space="PSUM") as ps_fo,
            ):
                wo_sb = []
                for dt in range(4):
                    w1 = pf_o.tile([P, D], bf16, name=f"wo{dt}", tag=f"wo{dt}", bufs=1)
                    nc.sync.dma_start(out=w1[:], in_=wo[dt * P : (dt + 1) * P, :])
                    wo_sb.append(w1)
                for it in range(NT):
                    for es in range(2):
                        pso = ps_fo.tile([P, 512], f32, tag="pso")
                        for dt in range(4):
                            nc.tensor.matmul(
                                pso[:],
                                yT_sb[dt][:, it * P : (it + 1) * P],
                                wo_sb[dt][:, es * 512 : (es + 1) * 512],
                                start=(dt == 0),
                                stop=(dt == 3),
                            )
                        osb = pf_o.tile([P, 512], f32, tag="osb")
                        nc.vector.tensor_copy(osb[:], pso[:])
                        nc.sync.dma_start(
                            out=out[it * P : (it + 1) * P, es * 512 : (es + 1) * 512],
                            in_=osb[:],
                        )

    nc.compile()
    return nc


def _prep_inputs(inputs):
    bf = ml_dtypes.bfloat16
    query, key_in, value = inputs["query"], inputs["key_in"], inputs["value"]
    pos_bias = inputs["pos_bias"]

    jj = np.arange(S)
    pbT = pos_bias.T.copy()  # [j, i]
    pbT[~(jj[:, None] <= jj[None, :] - (W - 1))] = 0.0
    pbT = pbT.astype(bf)

    wq_t = np.ascontiguousarray(inputs["Wq"].T).astype(bf)  # [din, e]
    wk_t = np.ascontiguousarray(inputs["Wk"].T).astype(bf)
    wv_t = np.ascontiguousarray(inputs["Wv"].T).astype(bf)
    wo_t = np.ascontiguousarray(inputs["Wo"].T).astype(bf)  # [d, e']

    keyT_b = [np.ascontiguousarray(key_in[:, b, :].T).astype(bf) for b in range(B)]
    valT_b = [np.ascontiguousarray(value[:, b, :].T).astype(bf) for b in range(B)]
    qT_b = [np.ascontiguousarray(query[:, b, :].T).astype(bf) for b in range(B)]

    in_maps = []
    for c in range(NC):
        b, h = c // 2, c % 2
        hs = slice(h * DH, (h + 1) * DH)
        in_maps.append(
            {
                "keyT": keyT_b[b],
                "valT": valT_b[b],
                "queryT": qT_b[b],
                "pbT": pbT,
                "wk": np.ascontiguousarray(wk_t[:, hs]),
                "wv": np.ascontiguousarray(wv_t[:, hs]),
                "wq": np.ascontiguousarray(wq_t[:, hs]),
                "wo": np.ascontiguousarray(wo_t[hs, :]),
            }
        )
    return in_maps


def _run(inputs, trace=False):
    from concourse.bass_utils import run_bass_kernel_spmd

    if "nc" not in _CACHE:
        _CACHE["nc"] = _build()
    nc = _CACHE["nc"]

    in_maps = _prep_inputs(inputs)
    res = run_bass_kernel_spmd(nc, in_maps, core_ids=list(range(NC)), trace=trace)

    # unshard: partial sums over d-halves per batch
    full = np.empty((S, B, D), np.float32)
    for b in range(B):
        p0 = np.asarray(res.results[2 * b]["out"], np.float32)
        p1 = np.asarray(res.results[2 * b + 1]["out"], np.float32)
        full[:, b, :] = p0 + p1
    return full, res


def kernel(**inputs):
    inputs = {k: np.asarray(v) for k, v in inputs.items()}
    full, _ = _run(inputs, trace=False)
    return full


if __name__ == "__main__":
    inputs = np.load("/tmp/inputs.npy", allow_pickle=True).item()
    out = kernel(**inputs)
    print("out", out.shape, out.dtype)


# revision 12
# speedup vs baseline: 2.1292x; 1.0937x over previous
"""AFT-Local distributed Trainium2 kernel (8 NeuronCores).

Math (reference, with cancellations):
  q = query @ Wq.T; k = key_in @ Wk.T; v = value @ Wv.T      [S,B,D]
  E[i,j] = exp(pos_bias[i,j] * (j <= i-255))                 [S,S]
  num[i,b,:] = sum_j E[i,j] * (exp(k)*v)[j,b,:]
  den[i,b,:] = sum_j E[i,j] *  exp(k)[j,b,:]
  out = (sigmoid(q) * num / den) @ Wo.T
The max-subtractions in the reference cancel in num/den; all values are small
enough that plain exp is safe.

Distribution (v4+): pure data/tensor-parallel, ZERO device collectives (the
collective control path on this fleet has a ~90us fixed cost, impossible to
hide). Core c owns (batch b = c//2, d-half h = c%2): it projects k/v/q for
all 2048 tokens restricted to its 512 d-columns (no duplicated FLOPs), runs
the full [2048x2048] E-weighted attention on its slice entirely out of SBUF,
and computes a PARTIAL output projection over its d-half. The host sums each
core-pair's f32 partials while unsharding - the only cross-core data motion
in the whole scheme.

Kernel structure (v5): all matmuls bf16 with f32 PSUM accumulation, in long
accumulation chains into a single PSUM bank (keeps the PE HAM-warm). The
attention num/den and the q projection run in the TRANSPOSED [d,i]
orientation so y comes out as y^T and feeds the output projection directly -
no on-chip transposes anywhere. The local mask is pre-applied to pos_bias^T
on the host (static index mask) so the device only exponentiates.
"""

import os
import sys

import numpy as np
import ml_dtypes

sys.path.insert(0, "/opt/trn_rl_repo")

S, B, D, W = 2048, 4, 1024, 256
NC = 8
P = 128
NT = S // P  # 16 token/row tiles
DH = 512  # d-half owned per core

_CACHE = {}


def _build():
    import concourse.bass as bass
    import concourse.bacc as bacc
    import concourse.mybir as mybir
    import concourse.tile as tile

    f32 = mybir.dt.float32
    bf16 = mybir.dt.bfloat16
    AF = mybir.ActivationFunctionType

    nc = bacc.Bacc("TRN2", target_bir_lowering=False, debug=False, num_devices=NC)

    # per-core inputs (b = batch owned, h = d-half owned)
    keyT = nc.dram_tensor("keyT", [D, S], bf16, kind="ExternalInput")  # key_in[:,b,:].T
    valT = nc.dram_tensor("valT", [D, S], bf16, kind="ExternalInput")
    queryT = nc.dram_tensor("queryT", [D, S], bf16, kind="ExternalInput")
    pbT = nc.dram_tensor("pbT", [S, S], bf16, kind="ExternalInput")  # masked pos_bias^T
    wk = nc.dram_tensor("wk", [D, DH], bf16, kind="ExternalInput")  # Wk.T[:, h-cols]
    wv = nc.dram_tensor("wv", [D, DH], bf16, kind="ExternalInput")
    wq = nc.dram_tensor("wq", [D, DH], bf16, kind="ExternalInput")
    wo = nc.dram_tensor("wo", [DH, D], bf16, kind="ExternalInput")  # Wo.T[h-rows, :]
    out = nc.dram_tensor("out", [S, D], f32, kind="ExternalOutput")  # partial!

    with tile.TileContext(nc) as tc:
        with tc.tile_pool(name="persist", bufs=1) as persist:
            # resident across phases (per-partition KB in comments)
            ek_sb = [persist.tile([P, DH], bf16, name=f"ek{t}") for t in range(NT)]    # 16
            ekv_sb = [persist.tile([P, DH], bf16, name=f"ekv{t}") for t in range(NT)]  # 16
            eT_sb = [persist.tile([P, S], bf16, name=f"eT{t}") for t in range(NT)]     # 64
            sqT_sb = [persist.tile([P, S], bf16, name=f"sqT{t}") for t in range(4)]    # 16
            yT_sb = [persist.tile([P, S], bf16, name=f"yT{t}") for t in range(4)]      # 16

            # ---- phase A: k/v projection (all tokens, own d-half), exp ----
            # two token-halves so keyT/valT are only half-resident
            with (
                tc.tile_pool(name="pa", bufs=1) as pa,
                tc.tile_pool(name="pa_st", bufs=3) as pa_st,
                tc.tile_pool(name="ps_a", bufs=2, space="PSUM") as ps_a,
            ):
                pd = tc.alloc_tile_pool(name="pd", bufs=3)
                wk_sb = []
                wv_sb = []
                for kt in range(8):
                    a3 = pa.tile([P, DH], bf16, name=f"wk{kt}")
                    nc.sync.dma_start(out=a3[:], in_=wk[kt * P : (kt + 1) * P, :])
                    wk_sb.append(a3)
                    a4 = pa.tile([P, DH], bf16, name=f"wv{kt}")
                    nc.sync.dma_start(out=a4[:], in_=wv[kt * P : (kt + 1) * P, :])
                    wv_sb.append(a4)
                # token quarters, double-buffered so loads prefetch ahead of
                # the WAR release; phase-D tiles interleave into the DMA gaps
                for q in range(4):
                    cs = slice(q * 512, (q + 1) * 512)
                    keyT_sb = []
                    valT_sb = []
                    for kt in range(8):
                        a1 = pa.tile(
                            [P, 512], bf16, tag=f"keyT{kt}", name=f"keyT{kt}", bufs=2
                        )
                        nc.sync.dma_start(out=a1[:], in_=keyT[kt * P : (kt + 1) * P, cs])
                        keyT_sb.append(a1)
                        a2 = pa.tile(
                            [P, 512], bf16, tag=f"valT{kt}", name=f"valT{kt}", bufs=2
                        )
                        nc.sync.dma_start(out=a2[:], in_=valT[kt * P : (kt + 1) * P, cs])
                        valT_sb.append(a2)
                    for jt in range(4 * q, 4 * q + 4):
                        pbt = pd.tile([P, S], bf16, tag="pbt")
                        nc.sync.dma_start(out=pbt[:], in_=pbT[jt * P : (jt + 1) * P, :])
                        nc.scalar.activation(eT_sb[jt][:], pbt[:], AF.Exp)
                    for tl in range(4):
                        tt = q * 4 + tl
                        psk = ps_a.tile([P, DH], f32, tag="psk")
                        psv = ps_a.tile([P, DH], f32, tag="psv")
                        for kt in range(8):
                            nc.tensor.matmul(
                                psk[:],
                                keyT_sb[kt][:, tl * P : (tl + 1) * P],
                                wk_sb[kt][:],
                                start=(kt == 0),
                                stop=(kt == 7),
                            )
                        for kt in range(8):
                            nc.tensor.matmul(
                                psv[:],
                                valT_sb[kt][:, tl * P : (tl + 1) * P],
                                wv_sb[kt][:],
                                start=(kt == 0),
                                stop=(kt == 7),
                            )
                        ekf = pa_st.tile([P, DH], f32, tag="ekf")
                        nc.scalar.activation(ekf[:], psk[:], AF.Exp)
                        nc.vector.tensor_copy(ek_sb[tt][:], ekf[:])
                        nc.vector.tensor_mul(ekv_sb[tt][:], ekf[:], psv[:])
                pd.release()

            # ---- phase C: q^T projection + sigmoid ([e,i] orientation) ----
            with (
                tc.tile_pool(name="pc", bufs=1) as pc,
                tc.tile_pool(name="ps_c", bufs=2, space="PSUM") as ps_c,
            ):
                wq_sb = []
                for kt in range(8):
                    c2 = pc.tile([P, DH], bf16, name=f"wq{kt}")
                    nc.sync.dma_start(out=c2[:], in_=wq[kt * P : (kt + 1) * P, :])
                    wq_sb.append(c2)
                for ib in range(4):
                    cs = slice(ib * 512, (ib + 1) * 512)
                    qT_sb = []
                    for kt in range(8):
                        c1 = pc.tile(
                            [P, 512], bf16, tag=f"qT{kt}", name=f"qT{kt}", bufs=2
                        )
                        nc.sync.dma_start(
                            out=c1[:], in_=queryT[kt * P : (kt + 1) * P, cs]
                        )
                        qT_sb.append(c1)
                    for et in range(4):
                        psq = ps_c.tile([P, 512], f32, tag="psq")
                        for kt in range(8):
                            nc.tensor.matmul(
                                psq[:],
                                wq_sb[kt][:, et * P : (et + 1) * P],
                                qT_sb[kt][:],
                                start=(kt == 0),
                                stop=(kt == 7),
                            )
                        nc.scalar.activation(
                            sqT_sb[et][:, ib * 512 : (ib + 1) * 512], psq[:], AF.Sigmoid
                        )

            # ---- phase E: num^T/den^T 16-chains in [d,i]; y^T epilogue ----
            with (
                tc.tile_pool(name="pe_ep", bufs=2) as pe_ep,
                tc.tile_pool(name="ps_e", bufs=2, space="PSUM") as ps_e,
            ):
                for ib in range(4):
                    csl = slice(ib * 512, (ib + 1) * 512)
                    for dt in range(4):
                        dsl = slice(dt * P, (dt + 1) * P)
                        na = ps_e.tile([P, 512], f32, tag="na")
                        da = ps_e.tile([P, 512], f32, tag="da")
                        for jt in range(NT):
                            nc.tensor.matmul(
                                na[:],
                                ekv_sb[jt][:, dsl],
                                eT_sb[jt][:, csl],
                                start=(jt == 0),
                                stop=(jt == NT - 1),
                            )
                        for jt in range(NT):
                            nc.tensor.matmul(
                                da[:],
                                ek_sb[jt][:, dsl],
                                eT_sb[jt][:, csl],
                                start=(jt == 0),
                                stop=(jt == NT - 1),
                            )
                        rec = pe_ep.tile([P, 512], f32, tag="rec")
                        nc.vector.reciprocal(rec[:], da[:])
                        tmp = pe_ep.tile([P, 512], f32, tag="tmp")
                        nc.vector.tensor_mul(tmp[:], na[:], rec[:])
                        nc.vector.tensor_mul(
                            yT_sb[dt][:, csl], tmp[:], sqT_sb[dt][:, csl]
                        )

            # ---- phase F: partial output projection (y^T is the lhsT) ----
            with (
                tc.tile_pool(name="pf_o", bufs=3) as pf_o,
                tc.tile_pool(name="ps_fo", bufs=2, space="PSUM") as ps_fo,
            ):
                wo_sb = []
                for dt in range(4):
                    w1 = pf_o.tile([P, D], bf16, name=f"wo{dt}", tag=f"wo{dt}", bufs=1)
                    nc.sync.dma_start(out=w1[:], in_=wo[dt * P : (dt + 1) * P, :])
                    wo_sb.append(w1)
                for it in range(NT):
                    for es in range(2):
                        pso = ps_fo.tile([P, 512], f32, tag="pso")
                        for dt in range(4):
                            nc.tensor.matmul(
                                pso[:],
                                yT_sb[dt][:, it * P : (it + 1) * P],
                                wo_sb[dt][:, es * 512 : (es + 1) * 512],
                                start=(dt == 0),
                                stop=(dt == 3),
                            )
                        osb = pf_o.tile([P, 512], f32, tag="osb")
                        nc.vector.tensor_copy(osb[:], pso[:])
                        nc.sync.dma_start(
                            out=out[it * P : (it + 1) * P, es * 512 : (es + 1) * 512],
                            in_=osb[:],
                        )

    nc.compile()
    return nc


def _prep_inputs(inputs):
    bf = ml_dtypes.bfloat16
    query, key_in, value = inputs["query"], inputs["key_in"], inputs["value"]
    pos_bias = inputs["pos_bias"]

    jj = np.arange(S)
    pbT = pos_bias.T.copy()  # [j, i]
    pbT[~(jj[:, None] <= jj[None, :] - (W - 1))] = 0.0
    pbT = pbT.astype(bf)

    wq_t = np.ascontiguousarray(inputs["Wq"].T).astype(bf)  # [din, e]
    wk_t = np.ascontiguousarray(inputs["Wk"].T).astype(bf)
    wv_t = np.ascontiguousarray(inputs["Wv"].T).astype(bf)
    wo_t = np.ascontiguousarray(inputs["Wo"].T).astype(bf)  # [d, e']

    keyT_b = [np.ascontiguousarray(key_in[:, b, :].T).astype(bf) for b in range(B)]
    valT_b = [np.ascontiguousarray(value[:, b, :].T).astype(bf) for b in range(B)]
    qT_b = [np.ascontiguousarray(query[:, b, :].T).astype(bf) for b in range(B)]

    in_maps = []
    for c in range(NC):
        b, h = c // 2, c % 2
        hs = slice(h * DH, (h + 1) * DH)
        in_maps.append(
            {
                "keyT": keyT_b[b],
                "valT": valT_b[b],
                "queryT": qT_b[b],
                "pbT": pbT,
                "wk": np.ascontiguousarray(wk_t[:, hs]),
                "wv": np.ascontiguousarray(wv_t[:, hs]),
                "wq": np.ascontiguousarray(wq_t[:, hs]),
                "wo": np.ascontiguousarray(wo_t[hs, :]),
            }
        )
    return in_maps


def _run(inputs, trace=False):
    from concourse.bass_utils import run_bass_kernel_spmd

    if "nc" not in _CACHE:
        _CACHE["nc"] = _build()
    nc = _CACHE["nc"]

    in_maps = _prep_inputs(inputs)
    res = run_bass_kernel_spmd(nc, in_maps, core_ids=list(range(NC)), trace=trace)

    # unshard: partial sums over d-halves per batch
    full = np.empty((S, B, D), np.float32)
    for b in range(B):
        p0 = np.asarray(res.results[2 * b]["out"], np.float32)
        p1 = np.asarray(res.results[2 * b + 1]["out"], np.float32)
        full[:, b, :] = p0 + p1
    return full, res


def kernel(**inputs):
    inputs = {k: np.asarray(v) for k, v in inputs.items()}
    full, _ = _run(inputs, trace=False)
    return full


if __name__ == "__main__":
    inputs = np.load("/tmp/inputs.npy", allow_pickle=True).item()
    out = kernel(**inputs)
    print("out", out.shape, out.dtype)
